# revision 46
# baseline (speedup 1.0000x reference)
"""Causal MHA forward on 8 NeuronCores (Trainium2, Bass/Tile).

Sharding: batch (4) x head-half (2) -> 8 cores. Each core computes, for its
batch b and its 8 heads: QKV column-sliced projections (bf16), causal
attention in transposed-score layout (S^T[k, q]), and a partial dense
projection against the matching 512-row slice of dense_w. The host sums the
two partial dense outputs per batch and adds dense_b + wv_b @ dense_w
(valid because softmax rows sum to 1).

Key layout tricks:
- Scores/PV computed per head-pair p (heads 2p, 2p+1). Head 2p's PV lhsT is
  [V_A | ones] -> psum rows 0:64 = O_A, row 64 = rowsum_A. Head 2p+1's lhsT
  is [ones | zeros*63 | V_B] -> psum row 0 = rowsum_B, rows 64:128 = O_B, so
  both heads' outputs land at their final partition ranges with no shift.
- Softmax normalization: reciprocals of the two rowsums -> one bf16 tile
  (rows 0 and 64), two 1-row broadcast matmuls fill a [128,512] psum with
  per-column reciprocals for both heads, then two DVE multiplies.
- Causal masking: diagonal-straddling 128-k-blocks restrict their q range
  to [off:512] (off = 0,128,256,256) and add a bf16 triangle/band mask via
  a small matmul; fully-masked blocks are never computed.
"""
import numpy as np
import ml_dtypes

import concourse.bacc as bacc
import concourse.bass as bass
import concourse.tile as tile
import concourse.mybir as mybir
from concourse.bass_utils import run_bass_kernel_spmd

B, S, D, = 4, 2048, 1024
DC = 512           # per-core d slice (8 heads x 64)
H = 8              # heads per core
DH = 64
N_CORES = 8
F32 = mybir.dt.float32
BF16 = mybir.dt.bfloat16
FP8 = mybir.dt.float8e4
DR = mybir.MatmulPerfMode.DoubleRow
AF = mybir.ActivationFunctionType
NEG = -1.0e9
# K/Q projections run in fp8 with weights pre-scaled by 32 (keeps them out of
# e4m3's subnormal range); scores are then (32*32)x too big, folded into the
# exp scale: 1/sqrt(d_model) / 1024
SCALE = 1.0 / 32.0 / 1024.0

# q-range starts for the 4 diagonal-straddling k-blocks of each 512-q chunk
OFFS = (0, 128, 256, 384)

_CACHE = {}


def _build():
    nc = bacc.Bacc("TRN2", target_bir_lowering=False, debug=False,
                   num_devices=N_CORES)
    xt = nc.dram_tensor("xt", [D, S], BF16, kind="ExternalInput")
    xt8 = nc.dram_tensor("xt8", [D, S], FP8, kind="ExternalInput")
    wq = nc.dram_tensor("wq", [D, DC], FP8, kind="ExternalInput")
    wk = nc.dram_tensor("wk", [D, DC], FP8, kind="ExternalInput")
    wv = nc.dram_tensor("wv", [D, DC], BF16, kind="ExternalInput")
    qb = nc.dram_tensor("qb", [DC], F32, kind="ExternalInput")
    kb = nc.dram_tensor("kb", [DC], F32, kind="ExternalInput")
    wd = nc.dram_tensor("wd", [DC, D], BF16, kind="ExternalInput")
    band = nc.dram_tensor("band", [128, 256], BF16, kind="ExternalInput")
    idm = nc.dram_tensor("idm", [128, 128], BF16, kind="ExternalInput")
    onb = nc.dram_tensor("onb", [128, 128], BF16, kind="ExternalInput")
    out = nc.dram_tensor("out", [S, D], F32, kind="ExternalOutput")

    with tile.TileContext(nc) as tc:
      with nc.allow_low_precision(reason="bf16 storage; all matmul accumulation in fp32 psum"):
        with (
            tc.tile_pool(name="consts", bufs=1) as consts,
            tc.tile_pool(name="ktp", bufs=1) as ktp,
            tc.tile_pool(name="vap", bufs=1) as vap,
            tc.tile_pool(name="otp", bufs=1) as otp,
            tc.tile_pool(name="qtp", bufs=1) as qtp,
            tc.tile_pool(name="xts", bufs=4) as xtsp,
            tc.tile_pool(name="ptp", bufs=2) as ptp,
            tc.tile_pool(name="nrm", bufs=2) as nrm,
            tc.tile_pool(name="psb", bufs=2, space="PSUM") as psb,
            tc.tile_pool(name="psv", bufs=1, space="PSUM") as psv,
            tc.tile_pool(name="psm", bufs=1, space="PSUM") as psm,
            tc.tile_pool(name="wts", bufs=1) as wkvp,
            tc.tile_pool(name="outp", bufs=3) as outp,
        ):
            band_sb = consts.tile([128, 256], BF16)
            nc.gpsimd.dma_start(out=band_sb, in_=band[:, :])
            id_sb = consts.tile([128, 128], BF16)
            nc.gpsimd.dma_start(out=id_sb, in_=idm[:, :])
            onb_sb = consts.tile([128, 128], BF16)
            nc.gpsimd.dma_start(out=onb_sb, in_=onb[:, :])
            qb_sb = consts.tile([128, 4], F32)
            nc.gpsimd.dma_start(out=qb_sb, in_=qb.ap().rearrange("(c p) -> p c", p=128))
            kb_sb = consts.tile([128, 4], F32)
            nc.gpsimd.dma_start(out=kb_sb, in_=kb.ap().rearrange("(c p) -> p c", p=128))

            kt = ktp.tile([128, 4, S], BF16)       # K^T, pair p rows = d 128p..
            qt = qtp.tile([128, 4, S], BF16)       # Q^T, full sequence
            # V per s-block & head pair: [V_A(64) | onesA | onesB | 0*63 | V_B(64)]
            va = vap.tile([128, 16, 4, 193], BF16)
            ot = otp.tile([128, 4, S], BF16)       # O^T accumulated
            nc.vector.memset(va[:, :, :, 64:66], 1.0)
            nc.vector.memset(va[:, :, :, 66:129], 0.0)

            # persistent normalization rhs: rows 0 (1/rowsum_B) and 64
            # (1/rowsum_A) are rewritten per head-pair; rows 1:64 stay zero so
            # a single K=65 broadcast matmul can read the whole tile
            rr2 = nrm.tile([65, 512], BF16, tag="rrP")
            nc.vector.memset(rr2, 0.0)

            wk_sb = wkvp.tile([128, 8, DC], FP8)
            wq_sb = wkvp.tile([128, 8, DC], FP8)
            wv_sb = wkvp.tile([128, 8, DC], BF16)
            wd_sb = wkvp.tile([128, 4, D], BF16)
            wkv = wk.ap().rearrange("(c p) d -> p c d", p=128)
            wqv = wq.ap().rearrange("(c p) d -> p c d", p=128)
            wvv = wv.ap().rearrange("(c p) d -> p c d", p=128)
            wdv = wd.ap().rearrange("(c p) d -> p c d", p=128)
            xv = xt.ap().rearrange("(i p) s -> p i s", p=128)
            x8v = xt8.ap().rearrange("(i p) s -> p i s", p=128)
            xs = {}
            x8s = {}
            for sc in range(4):
                xs[sc] = xtsp.tile([128, 8, 512], BF16, tag="xts", name=f"xts{sc}")
                x8s[sc] = xtsp.tile([128, 8, 512], FP8, tag="xts8",
                                    name=f"x8ts{sc}")

            # sliced loads so the first matmuls start early; wk/wq on the
            # scalar queue, x8 chunk 0 then wv/wd then x8 rest on Pool,
            # bf16 x on SP
            for lo, hi in ((0, 2), (2, 4), (4, 8)):
                nc.scalar.dma_start(out=x8s[0][:, lo:hi, :],
                                    in_=x8v[:, lo:hi, 0:512])
                nc.scalar.dma_start(out=wk_sb[:, lo:hi, :],
                                    in_=wkv[:, lo:hi, :])
            for h in range(2):
                nc.gpsimd.dma_start(out=wv_sb[:, 4 * h:4 * h + 4, :],
                                    in_=wvv[:, 4 * h:4 * h + 4, :])
            for sc in range(1, 4):
                nc.gpsimd.dma_start(out=x8s[sc],
                                    in_=x8v[:, :, 512 * sc:512 * (sc + 1)])
            nc.gpsimd.dma_start(out=wd_sb, in_=wdv)
            for sc in range(4):
                x = xs[sc]
                nsl = 4 if sc == 0 else 2
                w = 8 // nsl
                for i in range(nsl):
                    nc.sync.dma_start(
                        out=x[:, w * i:w * i + w, :],
                        in_=xv[:, w * i:w * i + w, 512 * sc:512 * (sc + 1)])
                if sc == 0:
                    for h in range(2):
                        nc.sync.dma_start(out=wq_sb[:, 4 * h:4 * h + 4, :],
                                          in_=wqv[:, 4 * h:4 * h + 4, :])

            # filler thunks: one matmul each, interleaved between attention
            # j-iterations so the (ACT-bound) attention stream never leaves
            # PE idle. A group's psum tile is allocated by its first thunk
            # and finalized (DVE drain) by its last.
            def group_thunks(make_mm, n_acc, finalize, name):
                box = []
                for i in range(n_acc):
                    def t(i=i, box=box):
                        if i == 0:
                            box.append(psm.tile([128, 512], F32, tag="mm",
                                                bufs=2, name=name))
                        make_mm(box[0], i)
                        if i == n_acc - 1:
                            finalize(box[0])
                    yield t

            def proj_thunks(sc):
                xg = xs[sc]
                x8g = x8s[sc]
                th = []
                for p in range(4):
                    th += list(group_thunks(
                        lambda ps, i, p=p: nc.tensor.matmul(
                            ps, wk_sb[:, 2 * i:2 * i + 2, 128 * p:128 * (p + 1)],
                            x8g[:, 2 * i:2 * i + 2, :], perf_mode=DR,
                            start=(i == 0), stop=(i == 3)),
                        4,
                        lambda ps, p=p: nc.vector.tensor_scalar_add(
                            out=kt[:, p, 512 * sc:512 * (sc + 1)], in0=ps,
                            scalar1=kb_sb[:, p:p + 1]),
                        "kps"))
                for p in range(4):
                    th += list(group_thunks(
                        lambda ps, i, p=p: nc.tensor.matmul(
                            ps, wq_sb[:, 2 * i:2 * i + 2, 128 * p:128 * (p + 1)],
                            x8g[:, 2 * i:2 * i + 2, :], perf_mode=DR,
                            start=(i == 0), stop=(i == 3)),
                        4,
                        lambda ps, p=p: nc.vector.tensor_scalar_add(
                            out=qt[:, p, 512 * sc:512 * (sc + 1)], in0=ps,
                            scalar1=qb_sb[:, p:p + 1]),
                        "qps"))
                for sb_ in range(4):
                    def vfin(ps, sb_=sb_):
                        sblk = 4 * sc + sb_
                        pv2 = ps.rearrange("s (pp two d) -> s pp two d", pp=4, two=2)
                        nc.vector.tensor_copy(out=va[:, sblk, :, 0:64],
                                              in_=pv2[:, :, 0, :])
                        nc.vector.tensor_copy(out=va[:, sblk, :, 129:193],
                                              in_=pv2[:, :, 1, :])
                    th += list(group_thunks(
                        lambda ps, i, sb_=sb_: nc.tensor.matmul(
                            ps, xg[:, i, 128 * sb_:128 * (sb_ + 1)], wv_sb[:, i, :],
                            start=(i == 0), stop=(i == 7)),
                        8, vfin, "vps"))
                return th

            def dense_thunks(cd):
                th = []
                for sb_ in range(4 * cd, 4 * cd + 4):
                    os_box = []
                    for n in range(2):
                        def mk(ps, i, n=n, sb_=sb_, os_box=os_box):
                            if n == 0 and i == 0:
                                os_box.append(outp.tile([128, 1024], F32, name="os"))
                            nc.tensor.matmul(ps, ot[:, i, 128 * sb_:128 * (sb_ + 1)],
                                             wd_sb[:, i, 512 * n:512 * (n + 1)],
                                             start=(i == 0), stop=(i == 3))
                        def dfin(ps, n=n, sb_=sb_, os_box=os_box):
                            nc.vector.tensor_copy(
                                out=os_box[0][:, 512 * n:512 * (n + 1)], in_=ps)
                            nc.sync.dma_start(
                                out=out[128 * sb_:128 * (sb_ + 1),
                                        512 * n:512 * (n + 1)],
                                in_=os_box[0][:, 512 * n:512 * (n + 1)])
                        th += list(group_thunks(mk, 4, dfin, "dps"))
                return th

            pending_norm = [None]

            def flush_norm():
                if pending_norm[0] is not None:
                    pending_norm[0]()
                    pending_norm[0] = None

            if True:
                # chunk-0 K projection: two groups i-interleaved so the first
                # matmuls keep pace with the piecewise x/wk DMA arrivals
                x80 = x8s[0]
                for pp in (0, 2):
                    psa = psm.tile([128, 512], F32, tag="mm", bufs=2, name="kps")
                    psb_ = psm.tile([128, 512], F32, tag="mm", bufs=2, name="kps")
                    for i in range(4):
                        nc.tensor.matmul(psa, wk_sb[:, 2 * i:2 * i + 2,
                                                    128 * pp:128 * (pp + 1)],
                                         x80[:, 2 * i:2 * i + 2, :], perf_mode=DR,
                                         start=(i == 0), stop=(i == 3))
                        nc.tensor.matmul(psb_, wk_sb[:, 2 * i:2 * i + 2,
                                                     128 * (pp + 1):128 * (pp + 2)],
                                         x80[:, 2 * i:2 * i + 2, :], perf_mode=DR,
                                         start=(i == 0), stop=(i == 3))
                    nc.vector.tensor_scalar_add(out=kt[:, pp, 0:512], in0=psa,
                                                scalar1=kb_sb[:, pp:pp + 1])
                    nc.vector.tensor_scalar_add(out=kt[:, pp + 1, 0:512], in0=psb_,
                                                scalar1=kb_sb[:, pp + 1:pp + 2])
                for t in proj_thunks(0)[16:]:
                    t()
                for c in range(4):
                    nj = 4 * c + 4
                    if c < 3:
                        filler = proj_thunks(c + 1)
                    else:
                        filler = (dense_thunks(0) + dense_thunks(1)
                                  + dense_thunks(2))
                    nf = len(filler)
                    # diagonal-straddling blocks first: the jj=0 (full-width)
                    # matmul must open every psum accumulation column group
                    order = list(range(4 * c, 4 * c + 4)) + list(range(4 * c))
                    for p in range(4):
                        filp = filler[nf * p // 4: nf * (p + 1) // 4]
                        # reserve a burst for the head-pair boundary, where PE
                        # waits on the last exp to free a score-psum buffer
                        nres = min(6, len(filp))
                        fil, res = filp[:len(filp) - nres], filp[len(filp) - nres:]
                        f0 = min(4, len(fil))
                        head, rest = fil[:f0], fil[f0:]
                        den2 = max(nj - 2, 1)

                        def fil_slice(idx, head=head, rest=rest, den2=den2,
                                      nj=nj):
                            if idx == 0:
                                return head
                            if idx >= nj - 1:
                                return []
                            k = idx - 1
                            return rest[len(rest) * k // den2:
                                        len(rest) * (k + 1) // den2]
                        pvA = psv.tile([65, 512], F32, tag="pvA", bufs=1, name="pvA")
                        pvB = psv.tile([128, 512], F32, tag="pvB", bufs=1, name="pvB")
                        for idx, j in enumerate(order):
                            jj = j - 4 * c
                            diag = jj >= 0
                            off = OFFS[jj] if diag else 0
                            first, last = idx == 0, idx == nj - 1
                            sc_ps = psb.tile([128, 1024], F32)
                            nc.tensor.matmul(sc_ps[:, off:512],
                                             kt[0:64, p, 128 * j:128 * (j + 1)],
                                             qt[0:64, p, 512 * c + off:512 * (c + 1)],
                                             start=True, stop=not diag)
                            nc.tensor.matmul(sc_ps[:, 512 + off:1024],
                                             kt[64:128, p, 128 * j:128 * (j + 1)],
                                             qt[64:128, p, 512 * c + off:512 * (c + 1)],
                                             start=True, stop=not diag)
                            if diag:
                                rh, tp0 = band_sb[:, 128:256], 128 * jj
                                nc.tensor.matmul(sc_ps[:, tp0:tp0 + 128], id_sb, rh,
                                                 start=False, stop=True)
                                nc.tensor.matmul(sc_ps[:, 512 + tp0:512 + tp0 + 128],
                                                 id_sb, rh, start=False, stop=True)
                            pt = ptp.tile([128, 1024], BF16)
                            if off:
                                sc3 = sc_ps.rearrange("p (h q) -> p h q", h=2)[:, :, off:512]
                                pt3 = pt.rearrange("p (h q) -> p h q", h=2)[:, :, off:512]
                                nc.scalar.activation(out=pt3, in_=sc3, func=AF.Exp,
                                                     scale=SCALE)
                            else:
                                nc.scalar.activation(out=pt, in_=sc_ps, func=AF.Exp,
                                                     scale=SCALE)
                            # filler between exp and PV: the PV pair waits on
                            # this j's exp, so PE covers the latency here
                            for t in (res if last else fil_slice(idx)):
                                t()
                            nc.tensor.matmul(pvA[:, off:512], va[:, j, p, 0:65],
                                             pt[:, off:512], start=first, stop=last)
                            nc.tensor.matmul(pvB[:, off:512], va[:, j, p, 65:193],
                                             pt[:, 512 + off:1024], start=first, stop=last)
                            if first:
                                # previous head-pair's bc+muls, deferred here
                                # so PE isn't stalled on the recip chain
                                flush_norm()
                        if c == 3 and p == 3:
                            # final pair: no successor needs the psum banks,
                            # so skip the staging copies to shorten the tail
                            nc.vector.reciprocal(out=rr2[64:65, :], in_=pvA[64:65, :])
                            nc.vector.reciprocal(out=rr2[0:1, :], in_=pvB[0:1, :])
                            bc = psm.tile([128, 512], F32, tag="mm", bufs=2,
                                          name="bc")
                            nc.tensor.matmul(bc, onb_sb[0:65, :], rr2[0:65, :],
                                             start=True, stop=True)
                            bcs = nrm.tile([128, 512], BF16, tag="pvBc")
                            # ACT is idle in the tail; keep DVE free for muls
                            nc.scalar.activation(out=bcs, in_=bc, func=AF.Copy,
                                                 scale=1.0)
                            nc.vector.tensor_mul(out=ot[0:64, p, 1536:2048],
                                                 in0=pvA[0:64, :], in1=bcs[0:64, :])
                            nc.vector.tensor_mul(out=ot[64:128, p, 1536:2048],
                                                 in0=pvB[64:128, :], in1=bcs[64:128, :])
                            continue
                        # normalization: rowsum_A at pvA[64], rowsum_B at pvB[0].
                        # Copy both psums to SBUF first so the banks free fast
                        # (next head-pair's PV matmuls reuse them), then work
                        # off the copies.
                        pvAc = nrm.tile([65, 512], BF16, tag="pvAc")
                        pvBc = nrm.tile([128, 512], BF16, tag="pvBc")
                        nc.vector.tensor_copy(out=pvAc, in_=pvA)
                        nc.vector.tensor_copy(out=pvBc, in_=pvB)
                        nc.vector.reciprocal(out=rr2[64:65, :], in_=pvAc[64:65, :])
                        nc.vector.reciprocal(out=rr2[0:1, :], in_=pvBc[0:1, :])

                        def do_norm(pvAc=pvAc, pvBc=pvBc, c=c, p=p):
                            bc = psm.tile([128, 512], F32, tag="mm", bufs=2,
                                          name="bc")
                            nc.tensor.matmul(bc, onb_sb[0:65, :], rr2[0:65, :],
                                             start=True, stop=True)
                            nc.vector.tensor_mul(
                                out=ot[0:64, p, 512 * c:512 * (c + 1)],
                                in0=pvAc[0:64, :], in1=bc[0:64, :])
                            nc.vector.tensor_mul(
                                out=ot[64:128, p, 512 * c:512 * (c + 1)],
                                in0=pvBc[64:128, :], in1=bc[64:128, :])
                        pending_norm[0] = do_norm
                flush_norm()
                # dense for the last chunk's 4 s-blocks
                for t in dense_thunks(3):
                    t()
    nc.compile()
    return nc


def get_nc():
    if "nc" not in _CACHE:
        _CACHE["nc"] = _build()
    return _CACHE["nc"]


def kernel(x, mask, wq_w, wq_b, wk_w, wk_b, wv_w, wv_b, dense_w, dense_b,
           _trace=False):
    bf = ml_dtypes.bfloat16
    x = np.asarray(x, dtype=np.float32)
    wq_w = np.asarray(wq_w, np.float32); wq_b = np.asarray(wq_b, np.float32)
    wk_w = np.asarray(wk_w, np.float32); wk_b = np.asarray(wk_b, np.float32)
    wv_w = np.asarray(wv_w, np.float32); wv_b = np.asarray(wv_b, np.float32)
    dense_w = np.asarray(dense_w, np.float32)
    dense_b = np.asarray(dense_b, np.float32)

    # causal masks, bf16: cols 0:128 = all -1e9; cols 128:256 = triangle
    # T[k, qq] = -1e9 where qq < k
    band = np.zeros((128, 256), np.float32)
    band[:, 0:128] = NEG
    k_idx = np.arange(128)[:, None]
    q_idx = np.arange(128)[None, :]
    band[:, 128:256] = np.where(q_idx < k_idx, NEG, 0.0)
    ident = np.eye(128, dtype=np.float32)
    onb = np.zeros((128, 128), np.float32)
    onb[64, 0:64] = 1.0   # broadcasts rowsum_A recip (at partition 64) to rows 0:64
    onb[0, 64:128] = 1.0  # broadcasts rowsum_B recip (at partition 0) to rows 64:128

    f8 = ml_dtypes.float8_e4m3fn
    in_maps = []
    for core in range(N_CORES):
        b, hh = divmod(core, 2)
        sl = slice(DC * hh, DC * (hh + 1))
        in_maps.append({
            "xt": np.ascontiguousarray(x[b].T).astype(bf),
            "xt8": np.ascontiguousarray(x[b].T).astype(f8),
            # K/Q weights and biases pre-scaled by 32 so fp8 weights stay
            # out of e4m3's subnormal range; folded back via the exp scale
            "wq": np.ascontiguousarray(wq_w[:, sl] * 32.0).astype(f8),
            "wk": np.ascontiguousarray(wk_w[:, sl] * 32.0).astype(f8),
            "wv": np.ascontiguousarray(wv_w[:, sl]).astype(bf),
            "qb": np.ascontiguousarray(wq_b[sl] * 32.0),
            "kb": np.ascontiguousarray(wk_b[sl] * 32.0),
            "wd": np.ascontiguousarray(dense_w[sl, :]).astype(bf),
            "band": band.astype(bf), "idm": ident.astype(bf),
            "onb": onb.astype(bf),
        })
    nc = get_nc()
    res = run_bass_kernel_spmd(nc, in_maps, core_ids=list(range(N_CORES)),
                               trace=_trace)
    const = dense_b + wv_b @ dense_w  # bias terms deferred to host
    outs = np.empty((B, S, D), np.float32)
    for b in range(B):
        outs[b] = res.results[2 * b]["out"] + res.results[2 * b + 1]["out"] + const
    if _trace:
        kernel.last_result = res
    return outs
